# revision 6
# baseline (speedup 1.0000x reference)
"""BitLinear int2 (ternary-weight) GEMM on 8 NeuronCores, fp8-hybrid.

out[8192, 16384] = (x[8192, 4096] @ w_q[16384, 4096].T) * gamma, fp16 I/O,
fp32 accumulation.

Strategy: tensor-parallel over out_features - each core gets a 2048-row
shard of w_q, x is replicated; host concatenates the 8 output shards.
The contraction is split: the first 2048 k-columns run as fp8(e4m3)
DoubleRow matmuls (2 fp8 weights per PE cell -> 256-deep contraction per
matmul, ~2x MAC rate), the last 2048 k-columns run exact fp16 matmuls.
The ternary weights are exact in e4m3; only x's fp8 half is quantized,
giving a measured 1.84e-2 relative error (gate 2e-2) at ~0.78x the
all-fp16 matmul count in PE cycles.

Both operands are host-transposed so the contraction dim lands on SBUF
partitions with plain (non-xbar) DMAs; x is host-packed per 256-token
superblock so loads are per-partition contiguous.  All weight shards
(4MB fp8 + 8MB fp16) stay resident in SBUF; x streams on the ACT HWDGE
ring while weights + outputs use the SP ring; K accumulates in PSUM
across 8 DoubleRow + 16 fp16 matmuls.  The first superblock interleaves
its two t-tiles k-outer across all 8 PSUM banks so the PE hides the
resident-weight fill; the last t-tile runs o-block-major so its copyback
trails by only one block.  gamma is baked into the PSUM->SBUF copy as an
immediate scale on the scalar engine.
"""

import sys

import ml_dtypes
import numpy as np

for _p in ("/opt/trn_rl_repo", "/root/.axon_site/_ro/trn_rl_repo"):
    if _p not in sys.path:
        sys.path.append(_p)

N_CORES = 8
N_TOKENS = 8192
IN_FEATURES = 4096
OUT_FEATURES = 16384
O_SHARD = OUT_FEATURES // N_CORES  # 2048

P = 128          # partitions / base matmul contraction tile
FREE = 512       # matmul moving free dim (one PSUM bank of fp32)
SB = 256         # tokens per x superblock (2 t-tiles)
KF8 = 2048       # leading k-columns in fp8 DoubleRow
KD8 = KF8 // (2 * P)       # 8 double-slabs (256 k each)
KF16 = IN_FEATURES - KF8   # trailing k-columns in fp16
KT16 = KF16 // P           # 16 k-slabs


def _build(gamma: float, T: int = N_TOKENS, O: int = O_SHARD, sb: int = SB):
    import concourse.mybir as mybir
    from concourse import bacc
    from concourse.tile import TileContext

    fp16 = mybir.dt.float16
    fp32 = mybir.dt.float32
    fp8 = mybir.dt.float8e4
    DR = mybir.MatmulPerfMode.DoubleRow

    NB = O // FREE     # 4 o-blocks per core
    TT = sb // P       # t-tiles per superblock
    NSB = T // sb      # superblocks

    nc = bacc.Bacc("TRN2", target_bir_lowering=False, debug=False,
                   num_devices=N_CORES)
    # x fp8 half: [p, s, d, i, t] = e4m3(x[s*sb+t, (2d+i)*128+p]); per
    # partition one superblock is 4KB contiguous.
    x8_d = nc.dram_tensor("x8", (P, NSB, KD8, 2, sb), fp8,
                          kind="ExternalInput")
    # x fp16 half: [p, s, k, t] = x[s*sb+t, KF8 + k*128 + p]
    x16_d = nc.dram_tensor("x16", (P, NSB, KT16, sb), fp16,
                           kind="ExternalInput")
    # w fp8 half: [d, p, i, o] = e4m3(w[o, (2d+i)*128+p])
    w8_d = nc.dram_tensor("w8", (KD8, P, 2, O), fp8, kind="ExternalInput")
    # w fp16-section weights: ternary, exact in e4m3; the moving operand of
    # a normal-mode matmul may be fp8 while the stationary x stays fp16.
    # Halves the resident-weight fill bytes (8MB vs 12MB).
    w16_d = nc.dram_tensor("w16", (KF16, O), fp8, kind="ExternalInput")
    out_d = nc.dram_tensor("out", (T, O), fp16, kind="ExternalOutput")

    with TileContext(nc) as tc:
        with tc.tile_pool(name="wpool", bufs=1) as wpool, \
             tc.tile_pool(name="x8pool", bufs=2) as x8pool, \
             tc.tile_pool(name="x16pool", bufs=2) as x16pool, \
             tc.tile_pool(name="opool", bufs=3) as opool, \
             tc.tile_pool(name="psum", bufs=8, space="PSUM") as psum_pool:

            # x loads ride the ACT HWDGE ring; weights + outputs ride the
            # SP ring so weight slab 0 is not queued behind x transfers.
            def load_x(s, eng):
                x8t = x8pool.tile([P, KD8, 2, sb], fp8, tag="x8",
                                  name=f"x8_{s}")
                if s == 0:
                    chunks = [(0, 1), (1, 2), (2, 4), (4, 6), (6, 8)]
                else:
                    chunks = [(2 * c, 2 * c + 2) for c in range(4)]
                for lo, hi in chunks:
                    eng.dma_start(out=x8t[:, lo:hi], in_=x8_d[:, s, lo:hi])
                x16t = x16pool.tile([P, KT16, sb], fp16, tag="x16",
                                    name=f"x16_{s}")
                for c in range(4):
                    eng.dma_start(out=x16t[:, 4 * c:4 * c + 4],
                                  in_=x16_d[:, s, 4 * c:4 * c + 4])
                return x8t, x16t

            # PE warm-up: a few matmuls on memset scratch keep the PE busy
            # from t~0 while the first weight/x DMAs land, so the HAM clock
            # gate reaches K=8/8 by ~3.5us instead of mid-superblock-0.
            wu_l = wpool.tile([P, P], fp16, name="wu_l")
            wu_r = wpool.tile([P, FREE], fp8, name="wu_r")
            nc.vector.memset(wu_l[:], 0.0)
            nc.vector.memset(wu_r[:], 0.0)
            wu_ps = psum_pool.tile([P, FREE], fp32, tag="ps", name="wu_ps")
            N_WARM = 6
            for i in range(N_WARM):
                nc.tensor.matmul(wu_ps, lhsT=wu_l[:], rhs=wu_r[:],
                                 start=(i == 0), stop=(i == N_WARM - 1))

            xts = {0: load_x(0, nc.scalar)}

            # Resident weights, one tile per slab so the first superblock's
            # k-loop paces along the arriving weight stream.  fp8 double-
            # slabs first (consumed first), then fp16-section slabs.  The
            # first double-slab lands in 4 o-chunks so the very first
            # matmul's dependency is only 128KB deep.
            w8s = []
            for d in range(KD8):
                wt = wpool.tile([P, 2, O], fp8, name=f"w8_{d}")
                if d == 0:
                    for c in range(4):
                        nc.sync.dma_start(
                            out=wt[:, :, c * FREE:(c + 1) * FREE],
                            in_=w8_d[d][:, :, c * FREE:(c + 1) * FREE])
                else:
                    nc.sync.dma_start(out=wt[:], in_=w8_d[d])
                w8s.append(wt)
            w16s = []
            for k in range(KT16):
                wt = wpool.tile([P, O], fp8, name=f"w16_{k}")
                nc.sync.dma_start(out=wt[:], in_=w16_d[k * P:(k + 1) * P, :])
                w16s.append(wt)

            xts[1] = load_x(1, nc.scalar)

            def dr_mm(ps, x8t, d, j, ob, start):
                nc.tensor.matmul(
                    ps,
                    lhsT=x8t[:, d, :, j * P:(j + 1) * P],
                    rhs=w8s[d][:, :, ob * FREE:(ob + 1) * FREE],
                    start=start, stop=False, perf_mode=DR)

            def f16_mm(ps, x16t, k, j, ob, stop):
                nc.tensor.matmul(
                    ps,
                    lhsT=x16t[:, k, j * P:(j + 1) * P],
                    rhs=w16s[k][:, ob * FREE:(ob + 1) * FREE],
                    start=False, stop=stop)

            def copyback(ot, psums, row):
                for ob in range(NB):
                    nc.scalar.mul(
                        out=ot[:, ob * FREE:(ob + 1) * FREE],
                        in_=psums[ob],
                        mul=gamma,
                    )
                nc.sync.dma_start(out=out_d[row:row + P, :], in_=ot)

            for s in range(NSB):
                t0 = s * sb
                if s not in xts:
                    xts[s] = load_x(s, nc.scalar)
                x8t, x16t = xts[s]

                if s == 0:
                    # Interleave both t-tiles k-outer: 8 matmuls per weight
                    # slab keeps the PE behind the DMA stream during the
                    # resident-weight fill.  Uses all 8 PSUM banks.
                    ots = [opool.tile([P, O], fp16, tag="ot", name=f"ot_0_{j}")
                           for j in range(TT)]
                    psums = [[psum_pool.tile([P, FREE], fp32, tag="ps",
                                             name=f"ps_0_{j}_{ob}")
                              for ob in range(NB)] for j in range(TT)]
                    for d in range(KD8):
                        for j in range(TT):
                            for ob in range(NB):
                                dr_mm(psums[j][ob], x8t, d, j, ob, d == 0)
                    for k in range(KT16):
                        for j in range(TT):
                            for ob in range(NB):
                                f16_mm(psums[j][ob], x16t, k, j, ob,
                                       k == KT16 - 1)
                    for j in range(TT):
                        copyback(ots[j], psums[j], t0 + j * P)
                else:
                    for j in range(TT):
                        ot = opool.tile([P, O], fp16, tag="ot",
                                        name=f"ot_{s}_{j}")
                        row = t0 + j * P
                        last = (s == NSB - 1 and j == TT - 1)
                        if last:
                            # o-block-major: each block's copy + store
                            # overlaps the next block's accumulation, so
                            # only one block's epilogue trails the PE.
                            for ob in range(NB):
                                ps = psum_pool.tile(
                                    [P, FREE], fp32, tag="ps",
                                    name=f"ps_{s}_{j}_{ob}")
                                for d in range(KD8):
                                    dr_mm(ps, x8t, d, j, ob, d == 0)
                                for k in range(KT16):
                                    f16_mm(ps, x16t, k, j, ob, k == KT16 - 1)
                                nc.scalar.mul(
                                    out=ot[:, ob * FREE:(ob + 1) * FREE],
                                    in_=ps,
                                    mul=gamma,
                                )
                                nc.sync.dma_start(
                                    out=out_d[row:row + P,
                                              ob * FREE:(ob + 1) * FREE],
                                    in_=ot[:, ob * FREE:(ob + 1) * FREE])
                            continue
                        psums = [psum_pool.tile([P, FREE], fp32, tag="ps",
                                                name=f"ps_{s}_{j}_{ob}")
                                 for ob in range(NB)]
                        for d in range(KD8):
                            for ob in range(NB):
                                dr_mm(psums[ob], x8t, d, j, ob, d == 0)
                        for k in range(KT16):
                            for ob in range(NB):
                                f16_mm(psums[ob], x16t, k, j, ob,
                                       k == KT16 - 1)
                        copyback(ot, psums, t0 + j * P)

    nc.compile()
    return nc


def _pack_inputs(x: np.ndarray, w: np.ndarray):
    """Host-side packing: quantize/transpose into the kernel layouts."""
    e4 = ml_dtypes.float8_e4m3fn
    NSB = N_TOKENS // SB
    # fp8 half of x: [tok, k] -> [p, s, d, i, t]
    a = x[:, :KF8].astype(e4).reshape(NSB, SB, KF8 // P, P)
    x8 = np.ascontiguousarray(a.transpose(3, 0, 2, 1)).reshape(
        P, NSB, KD8, 2, SB)
    # fp16 half of x: [tok, k] -> [p, s, k, t]
    b = x[:, KF8:].reshape(NSB, SB, KT16, P)
    x16 = np.ascontiguousarray(b.transpose(3, 0, 2, 1))
    # per-core weight shards
    w8s, w16s = [], []
    for c in range(N_CORES):
        wc = w[c * O_SHARD:(c + 1) * O_SHARD, :]  # [o, k]
        v = np.ascontiguousarray(wc[:, :KF8].T).reshape(KD8, 2, P, O_SHARD)
        w8s.append(np.ascontiguousarray(
            v.transpose(0, 2, 1, 3)).astype(e4))
        w16s.append(np.ascontiguousarray(wc[:, KF8:].T).astype(e4))
    return x8, x16, w8s, w16s


def _run(inputs, trace=False):
    import os

    from concourse.bass_utils import run_bass_kernel_spmd

    if not trace:
        # A stray BASS_TRACE would route run_bass_kernel_spmd into the NTFF
        # hook import, which this container lacks.
        os.environ["BASS_NEVER_TRACE"] = "1"
    else:
        os.environ.pop("BASS_NEVER_TRACE", None)

    x = np.asarray(inputs["x"])
    w = np.asarray(inputs["w_q"])
    gamma = float(np.asarray(inputs["gamma"]).astype(np.float32).reshape(-1)[0])

    x8, x16, w8s, w16s = _pack_inputs(x, w)
    nc = _build(gamma)
    in_maps = []
    for c in range(N_CORES):
        in_maps.append({"x8": x8, "x16": x16, "w8": w8s[c], "w16": w16s[c]})

    res = run_bass_kernel_spmd(nc, in_maps, core_ids=list(range(N_CORES)),
                               trace=trace)
    out = np.concatenate(
        [np.asarray(res.results[c]["out"]) for c in range(N_CORES)], axis=1)
    return out.astype(np.float16, copy=False), res


def kernel(**inputs) -> np.ndarray:
    out, _ = _run(inputs, trace=False)
    return out


# revision 8
# speedup vs baseline: 1.0045x; 1.0045x over previous
"""BitLinear int2 (ternary-weight) GEMM on 8 NeuronCores, fp8-hybrid.

out[8192, 16384] = (x[8192, 4096] @ w_q[16384, 4096].T) * gamma, fp16 I/O,
fp32 accumulation.

Strategy: tensor-parallel over out_features - each core gets a 2048-row
shard of w_q, x is replicated; host concatenates the 8 output shards.
The contraction is split: the first 2048 k-columns run as fp8(e4m3)
DoubleRow matmuls (2 fp8 weights per PE cell -> 256-deep contraction per
matmul, ~2x MAC rate), the last 2048 k-columns run exact fp16 matmuls.
The ternary weights are exact in e4m3; only x's fp8 half is quantized,
giving a measured 1.84e-2 relative error (gate 2e-2) at ~0.78x the
all-fp16 matmul count in PE cycles.

Both operands are host-transposed so the contraction dim lands on SBUF
partitions with plain (non-xbar) DMAs; x is host-packed per 256-token
superblock so loads are per-partition contiguous.  All weight shards
(4MB fp8 + 8MB fp16) stay resident in SBUF; x streams on the ACT HWDGE
ring while weights + outputs use the SP ring; K accumulates in PSUM
across 8 DoubleRow + 16 fp16 matmuls.  The first superblock interleaves
its two t-tiles k-outer across all 8 PSUM banks so the PE hides the
resident-weight fill; the last t-tile runs o-block-major so its copyback
trails by only one block.  gamma is baked into the PSUM->SBUF copy as an
immediate scale on the scalar engine.
"""

import sys

import ml_dtypes
import numpy as np

for _p in ("/opt/trn_rl_repo", "/root/.axon_site/_ro/trn_rl_repo"):
    if _p not in sys.path:
        sys.path.append(_p)

N_CORES = 8
N_TOKENS = 8192
IN_FEATURES = 4096
OUT_FEATURES = 16384
O_SHARD = OUT_FEATURES // N_CORES  # 2048

P = 128          # partitions / base matmul contraction tile
FREE = 512       # matmul moving free dim (one PSUM bank of fp32)
SB = 256         # tokens per x superblock (2 t-tiles)
KF8 = 2048       # leading k-columns in fp8 DoubleRow
KD8 = KF8 // (2 * P)       # 8 double-slabs (256 k each)
KF16 = IN_FEATURES - KF8   # trailing k-columns in fp16
KT16 = KF16 // P           # 16 k-slabs


def _build(gamma: float, T: int = N_TOKENS, O: int = O_SHARD, sb: int = SB):
    import concourse.mybir as mybir
    from concourse import bacc
    from concourse.tile import TileContext

    fp16 = mybir.dt.float16
    fp32 = mybir.dt.float32
    fp8 = mybir.dt.float8e4
    DR = mybir.MatmulPerfMode.DoubleRow

    NB = O // FREE     # 4 o-blocks per core
    TT = sb // P       # t-tiles per superblock
    NSB = T // sb      # superblocks

    nc = bacc.Bacc("TRN2", target_bir_lowering=False, debug=False,
                   num_devices=N_CORES)
    # x fp8 half: [p, s, d, i, t] = e4m3(x[s*sb+t, (2d+i)*128+p]); per
    # partition one superblock is 4KB contiguous.
    x8_d = nc.dram_tensor("x8", (P, NSB, KD8, 2, sb), fp8,
                          kind="ExternalInput")
    # x fp16 half: [p, s, k, t] = x[s*sb+t, KF8 + k*128 + p]
    x16_d = nc.dram_tensor("x16", (P, NSB, KT16, sb), fp16,
                           kind="ExternalInput")
    # w fp8 half: [d, p, i, o] = e4m3(w[o, (2d+i)*128+p])
    w8_d = nc.dram_tensor("w8", (KD8, P, 2, O), fp8, kind="ExternalInput")
    # w fp16-section weights: ternary, exact in e4m3; the moving operand of
    # a normal-mode matmul may be fp8 while the stationary x stays fp16.
    # Halves the resident-weight fill bytes (8MB vs 12MB).
    w16_d = nc.dram_tensor("w16", (KF16, O), fp8, kind="ExternalInput")
    out_d = nc.dram_tensor("out", (T, O), fp16, kind="ExternalOutput")

    with TileContext(nc) as tc:
        with tc.tile_pool(name="wpool", bufs=1) as wpool, \
             tc.tile_pool(name="x8pool", bufs=2) as x8pool, \
             tc.tile_pool(name="x16pool", bufs=2) as x16pool, \
             tc.tile_pool(name="opool", bufs=3) as opool, \
             tc.tile_pool(name="psum", bufs=8, space="PSUM") as psum_pool:

            # x loads ride the ACT HWDGE ring; weights + outputs ride the
            # SP ring so weight slab 0 is not queued behind x transfers.
            def load_x(s, eng):
                x8t = x8pool.tile([P, KD8, 2, sb], fp8, tag="x8",
                                  name=f"x8_{s}")
                if s == 0:
                    chunks = [(0, 1), (1, 2), (2, 4), (4, 6), (6, 8)]
                else:
                    chunks = [(2 * c, 2 * c + 2) for c in range(4)]
                for lo, hi in chunks:
                    eng.dma_start(out=x8t[:, lo:hi], in_=x8_d[:, s, lo:hi])
                x16t = x16pool.tile([P, KT16, sb], fp16, tag="x16",
                                    name=f"x16_{s}")
                for c in range(4):
                    eng.dma_start(out=x16t[:, 4 * c:4 * c + 4],
                                  in_=x16_d[:, s, 4 * c:4 * c + 4])
                return x8t, x16t

            # PE warm-up: a few matmuls on memset scratch keep the PE busy
            # from t~0 while the first weight/x DMAs land, so the HAM clock
            # gate reaches K=8/8 by ~3.5us instead of mid-superblock-0.
            wu_l = wpool.tile([P, P], fp16, name="wu_l")
            wu_r = wpool.tile([P, FREE], fp8, name="wu_r")
            nc.gpsimd.memset(wu_l[:], 0.0)
            nc.gpsimd.memset(wu_r[:], 0.0)
            wu_ps = psum_pool.tile([P, FREE], fp32, tag="ps", name="wu_ps")
            N_WARM = 8
            for i in range(N_WARM):
                nc.tensor.matmul(wu_ps, lhsT=wu_l[:], rhs=wu_r[:],
                                 start=(i == 0), stop=(i == N_WARM - 1))

            xts = {0: load_x(0, nc.scalar)}

            # Resident weights, one tile per slab so the first superblock's
            # k-loop paces along the arriving weight stream.  fp8 double-
            # slabs first (consumed first), then fp16-section slabs.  The
            # first double-slab lands in 4 o-chunks so the very first
            # matmul's dependency is only 128KB deep.
            w8s = []
            for d in range(KD8):
                wt = wpool.tile([P, 2, O], fp8, name=f"w8_{d}")
                if d == 0:
                    for c in range(4):
                        nc.sync.dma_start(
                            out=wt[:, :, c * FREE:(c + 1) * FREE],
                            in_=w8_d[d][:, :, c * FREE:(c + 1) * FREE])
                else:
                    nc.sync.dma_start(out=wt[:], in_=w8_d[d])
                w8s.append(wt)
            w16s = []
            for k in range(KT16):
                wt = wpool.tile([P, O], fp8, name=f"w16_{k}")
                nc.sync.dma_start(out=wt[:], in_=w16_d[k * P:(k + 1) * P, :])
                w16s.append(wt)

            # Superblock 1 queues on the SP ring *behind* the weight stream:
            # it isn't needed until ~43us and must not steal HBM bandwidth
            # from the resident-weight fill (measured: on the ACT ring it
            # runs at ~15-30us and stalls the w16 stream mid-superblock-0).
            xts[1] = load_x(1, nc.sync)

            def dr_mm(ps, x8t, d, j, ob, start):
                nc.tensor.matmul(
                    ps,
                    lhsT=x8t[:, d, :, j * P:(j + 1) * P],
                    rhs=w8s[d][:, :, ob * FREE:(ob + 1) * FREE],
                    start=start, stop=False, perf_mode=DR)

            def f16_mm(ps, x16t, k, j, ob, stop):
                nc.tensor.matmul(
                    ps,
                    lhsT=x16t[:, k, j * P:(j + 1) * P],
                    rhs=w16s[k][:, ob * FREE:(ob + 1) * FREE],
                    start=False, stop=stop)

            def copyback(ot, psums, row):
                for ob in range(NB):
                    nc.scalar.mul(
                        out=ot[:, ob * FREE:(ob + 1) * FREE],
                        in_=psums[ob],
                        mul=gamma,
                    )
                nc.sync.dma_start(out=out_d[row:row + P, :], in_=ot)

            for s in range(NSB):
                t0 = s * sb
                if s not in xts:
                    xts[s] = load_x(s, nc.scalar)
                x8t, x16t = xts[s]

                if s == 0:
                    # Interleave both t-tiles k-outer: 8 matmuls per weight
                    # slab keeps the PE behind the DMA stream during the
                    # resident-weight fill.  Uses all 8 PSUM banks.
                    ots = [opool.tile([P, O], fp16, tag="ot", name=f"ot_0_{j}")
                           for j in range(TT)]
                    psums = [[psum_pool.tile([P, FREE], fp32, tag="ps",
                                             name=f"ps_0_{j}_{ob}")
                              for ob in range(NB)] for j in range(TT)]
                    for d in range(KD8):
                        for j in range(TT):
                            for ob in range(NB):
                                dr_mm(psums[j][ob], x8t, d, j, ob, d == 0)
                    for k in range(KT16):
                        for j in range(TT):
                            for ob in range(NB):
                                f16_mm(psums[j][ob], x16t, k, j, ob,
                                       k == KT16 - 1)
                    for j in range(TT):
                        copyback(ots[j], psums[j], t0 + j * P)
                else:
                    for j in range(TT):
                        ot = opool.tile([P, O], fp16, tag="ot",
                                        name=f"ot_{s}_{j}")
                        row = t0 + j * P
                        last = (s == NSB - 1 and j == TT - 1)
                        if last:
                            # o-block-major: each block's copy + store
                            # overlaps the next block's accumulation, so
                            # only one block's epilogue trails the PE.
                            for ob in range(NB):
                                ps = psum_pool.tile(
                                    [P, FREE], fp32, tag="ps",
                                    name=f"ps_{s}_{j}_{ob}")
                                for d in range(KD8):
                                    dr_mm(ps, x8t, d, j, ob, d == 0)
                                for k in range(KT16):
                                    f16_mm(ps, x16t, k, j, ob, k == KT16 - 1)
                                nc.scalar.mul(
                                    out=ot[:, ob * FREE:(ob + 1) * FREE],
                                    in_=ps,
                                    mul=gamma,
                                )
                                nc.sync.dma_start(
                                    out=out_d[row:row + P,
                                              ob * FREE:(ob + 1) * FREE],
                                    in_=ot[:, ob * FREE:(ob + 1) * FREE])
                            continue
                        psums = [psum_pool.tile([P, FREE], fp32, tag="ps",
                                                name=f"ps_{s}_{j}_{ob}")
                                 for ob in range(NB)]
                        for d in range(KD8):
                            for ob in range(NB):
                                dr_mm(psums[ob], x8t, d, j, ob, d == 0)
                        for k in range(KT16):
                            for ob in range(NB):
                                f16_mm(psums[ob], x16t, k, j, ob,
                                       k == KT16 - 1)
                        copyback(ot, psums, t0 + j * P)

    nc.compile()
    return nc


def _pack_inputs(x: np.ndarray, w: np.ndarray):
    """Host-side packing: quantize/transpose into the kernel layouts."""
    e4 = ml_dtypes.float8_e4m3fn
    NSB = N_TOKENS // SB
    # fp8 half of x: [tok, k] -> [p, s, d, i, t]
    a = x[:, :KF8].astype(e4).reshape(NSB, SB, KF8 // P, P)
    x8 = np.ascontiguousarray(a.transpose(3, 0, 2, 1)).reshape(
        P, NSB, KD8, 2, SB)
    # fp16 half of x: [tok, k] -> [p, s, k, t]
    b = x[:, KF8:].reshape(NSB, SB, KT16, P)
    x16 = np.ascontiguousarray(b.transpose(3, 0, 2, 1))
    # per-core weight shards
    w8s, w16s = [], []
    for c in range(N_CORES):
        wc = w[c * O_SHARD:(c + 1) * O_SHARD, :]  # [o, k]
        v = np.ascontiguousarray(wc[:, :KF8].T).reshape(KD8, 2, P, O_SHARD)
        w8s.append(np.ascontiguousarray(
            v.transpose(0, 2, 1, 3)).astype(e4))
        w16s.append(np.ascontiguousarray(wc[:, KF8:].T).astype(e4))
    return x8, x16, w8s, w16s


def _run(inputs, trace=False):
    import os

    from concourse.bass_utils import run_bass_kernel_spmd

    if not trace:
        # A stray BASS_TRACE would route run_bass_kernel_spmd into the NTFF
        # hook import, which this container lacks.
        os.environ["BASS_NEVER_TRACE"] = "1"
    else:
        os.environ.pop("BASS_NEVER_TRACE", None)

    x = np.asarray(inputs["x"])
    w = np.asarray(inputs["w_q"])
    gamma = float(np.asarray(inputs["gamma"]).astype(np.float32).reshape(-1)[0])

    x8, x16, w8s, w16s = _pack_inputs(x, w)
    nc = _build(gamma)
    in_maps = []
    for c in range(N_CORES):
        in_maps.append({"x8": x8, "x16": x16, "w8": w8s[c], "w16": w16s[c]})

    res = run_bass_kernel_spmd(nc, in_maps, core_ids=list(range(N_CORES)),
                               trace=trace)
    out = np.concatenate(
        [np.asarray(res.results[c]["out"]) for c in range(N_CORES)], axis=1)
    return out.astype(np.float16, copy=False), res


def kernel(**inputs) -> np.ndarray:
    out, _ = _run(inputs, trace=False)
    return out


# revision 16
# speedup vs baseline: 1.0047x; 1.0002x over previous
"""BitLinear int2 (ternary-weight) GEMM on 8 NeuronCores, fp8-hybrid.

out[8192, 16384] = (x[8192, 4096] @ w_q[16384, 4096].T) * gamma, fp16 I/O,
fp32 accumulation.

Strategy: tensor-parallel over out_features - each core gets a 2048-row
shard of w_q, x is replicated; host concatenates the 8 output shards.
The contraction is split: the first 2048 k-columns run as fp8(e4m3)
DoubleRow matmuls (2 fp8 weights per PE cell -> 256-deep contraction per
matmul, ~2x MAC rate), the last 2048 k-columns run exact fp16 matmuls.
The ternary weights are exact in e4m3; only x's fp8 half is quantized,
giving a measured 1.84e-2 relative error (gate 2e-2) at ~0.78x the
all-fp16 matmul count in PE cycles.

Both operands are host-transposed so the contraction dim lands on SBUF
partitions with plain (non-xbar) DMAs; x is host-packed per 256-token
superblock so loads are per-partition contiguous.  All weight shards
(4MB fp8 + 8MB fp16) stay resident in SBUF; x streams on the ACT HWDGE
ring while weights + outputs use the SP ring; K accumulates in PSUM
across 8 DoubleRow + 16 fp16 matmuls.  The first superblock interleaves
its two t-tiles k-outer across all 8 PSUM banks so the PE hides the
resident-weight fill; the last t-tile runs o-block-major so its copyback
trails by only one block.  gamma is baked into the PSUM->SBUF copy as an
immediate scale on the scalar engine.
"""

import sys

import ml_dtypes
import numpy as np

for _p in ("/opt/trn_rl_repo", "/root/.axon_site/_ro/trn_rl_repo"):
    if _p not in sys.path:
        sys.path.append(_p)

N_CORES = 8
N_TOKENS = 8192
IN_FEATURES = 4096
OUT_FEATURES = 16384
O_SHARD = OUT_FEATURES // N_CORES  # 2048

P = 128          # partitions / base matmul contraction tile
FREE = 512       # matmul moving free dim (one PSUM bank of fp32)
SB = 256         # tokens per x superblock (2 t-tiles)
KF8 = 2048       # leading k-columns in fp8 DoubleRow
KD8 = KF8 // (2 * P)       # 8 double-slabs (256 k each)
KF16 = IN_FEATURES - KF8   # trailing k-columns in fp16
KT16 = KF16 // P           # 16 k-slabs


def _build(gamma: float, T: int = N_TOKENS, O: int = O_SHARD, sb: int = SB):
    import concourse.mybir as mybir
    from concourse import bacc
    from concourse.tile import TileContext

    fp16 = mybir.dt.float16
    fp32 = mybir.dt.float32
    fp8 = mybir.dt.float8e4
    DR = mybir.MatmulPerfMode.DoubleRow

    NB = O // FREE     # 4 o-blocks per core
    TT = sb // P       # t-tiles per superblock
    NSB = T // sb      # superblocks

    nc = bacc.Bacc("TRN2", target_bir_lowering=False, debug=False,
                   num_devices=N_CORES)
    # x fp8 half: [p, s, d, i, t] = e4m3(x[s*sb+t, (2d+i)*128+p]); per
    # partition one superblock is 4KB contiguous.
    x8_d = nc.dram_tensor("x8", (P, NSB, KD8, 2, sb), fp8,
                          kind="ExternalInput")
    # x fp16 half: [p, s, k, t] = x[s*sb+t, KF8 + k*128 + p]
    x16_d = nc.dram_tensor("x16", (P, NSB, KT16, sb), fp16,
                           kind="ExternalInput")
    # w fp8 half: [p, d, i, o] = e4m3(w[o, (2d+i)*128+p])
    w8_d2 = nc.dram_tensor("w8", (P, KD8, 2, O), fp8, kind="ExternalInput")
    # w fp16-section weights: ternary, exact in e4m3; the moving operand of
    # a normal-mode matmul may be fp8 while the stationary x stays fp16.
    # Halves the resident-weight fill bytes (8MB vs 12MB).
    # layout [p, k, o] = w[o, KF8 + k*128 + p]
    w16_d2 = nc.dram_tensor("w16", (P, KT16, O), fp8, kind="ExternalInput")
    out_d = nc.dram_tensor("out", (T, O), fp16, kind="ExternalOutput")

    with TileContext(nc) as tc:
        with tc.tile_pool(name="wpool", bufs=1) as wpool, \
             tc.tile_pool(name="x8pool", bufs=2) as x8pool, \
             tc.tile_pool(name="x16pool", bufs=2) as x16pool, \
             tc.tile_pool(name="opool", bufs=3) as opool, \
             tc.tile_pool(name="psum", bufs=8, space="PSUM") as psum_pool:

            # x loads ride the ACT HWDGE ring; weights + outputs ride the
            # SP ring so weight slab 0 is not queued behind x transfers.
            def load_x(s, eng):
                x8t = x8pool.tile([P, KD8, 2, sb], fp8, tag="x8",
                                  name=f"x8_{s}")
                if s == 0:
                    chunks = [(0, 1), (1, 2), (2, 4), (4, 6), (6, 8)]
                else:
                    chunks = [(0, 4), (4, 8)]
                for lo, hi in chunks:
                    eng.dma_start(out=x8t[:, lo:hi], in_=x8_d[:, s, lo:hi])
                x16t = x16pool.tile([P, KT16, sb], fp16, tag="x16",
                                    name=f"x16_{s}")
                nch = 4 if s == 0 else 2
                w = KT16 // nch
                for c in range(nch):
                    eng.dma_start(out=x16t[:, w * c:w * c + w],
                                  in_=x16_d[:, s, w * c:w * c + w])
                return x8t, x16t

            # PE warm-up: a few matmuls on memset scratch keep the PE busy
            # from t~0 while the first weight/x DMAs land, so the HAM clock
            # gate reaches K=8/8 by ~3.5us instead of mid-superblock-0.
            wu_l = wpool.tile([P, P], fp16, name="wu_l")
            wu_r = wpool.tile([P, FREE], fp8, name="wu_r")
            nc.gpsimd.memset(wu_l[:], 0.0)
            nc.gpsimd.memset(wu_r[:], 0.0)
            wu_ps = psum_pool.tile([P, FREE], fp32, tag="ps", name="wu_ps")
            N_WARM = 3
            for i in range(N_WARM):
                nc.tensor.matmul(wu_ps, lhsT=wu_l[:], rhs=wu_r[:],
                                 start=(i == 0), stop=(i == N_WARM - 1))

            xts = {0: load_x(0, nc.scalar)}

            # Resident weights in two mega-tiles loaded by a handful of
            # large DMAs: descriptor generation on the sync engine costs
            # ~600ns per dma_start, so many small slab loads starve the
            # early stream.  Region-granular tile deps still let the first
            # superblock's k-loop pace along the arriving chunks.  fp8
            # double-slabs first (consumed first), then fp16-section slabs.
            w8m = wpool.tile([P, KD8, 2, O], fp8, name="w8m")
            for lo, hi in [(0, 1), (1, 2), (2, 4), (4, 6), (6, 8)]:
                nc.sync.dma_start(out=w8m[:, lo:hi], in_=w8_d2[:, lo:hi])
            w16m = wpool.tile([P, KT16, O], fp8, name="w16m")
            for c in range(4):
                nc.sync.dma_start(out=w16m[:, 4 * c:4 * c + 4],
                                  in_=w16_d2[:, 4 * c:4 * c + 4])

            # Superblock 1 queues on the SP ring *behind* the weight stream:
            # it isn't needed until ~43us and must not steal HBM bandwidth
            # from the resident-weight fill (measured: on the ACT ring it
            # runs at ~15-30us and stalls the w16 stream mid-superblock-0).
            xts[1] = load_x(1, nc.sync)

            def dr_mm(ps, x8t, d, j, ob, start, free=FREE):
                nc.tensor.matmul(
                    ps,
                    lhsT=x8t[:, d, :, j * P:(j + 1) * P],
                    rhs=w8m[:, d, :, ob * free:(ob + 1) * free],
                    start=start, stop=False, perf_mode=DR)

            def f16_mm(ps, x16t, k, j, ob, stop, free=FREE):
                nc.tensor.matmul(
                    ps,
                    lhsT=x16t[:, k, j * P:(j + 1) * P],
                    rhs=w16m[:, k, ob * free:(ob + 1) * free],
                    start=False, stop=stop)

            def copyback(ot, psums, row):
                for ob in range(NB):
                    nc.scalar.mul(
                        out=ot[:, ob * FREE:(ob + 1) * FREE],
                        in_=psums[ob],
                        mul=gamma,
                    )
                nc.sync.dma_start(out=out_d[row:row + P, :], in_=ot)

            for s in range(NSB):
                t0 = s * sb
                if s not in xts:
                    xts[s] = load_x(s, nc.scalar)
                x8t, x16t = xts[s]

                if s == 0:
                    # Interleave both t-tiles k-outer: 8 matmuls per weight
                    # slab keeps the PE behind the DMA stream during the
                    # resident-weight fill.  Uses all 8 PSUM banks.
                    ots = [opool.tile([P, O], fp16, tag="ot", name=f"ot_0_{j}")
                           for j in range(TT)]
                    psums = [[psum_pool.tile([P, FREE], fp32, tag="ps",
                                             name=f"ps_0_{j}_{ob}")
                              for ob in range(NB)] for j in range(TT)]
                    for d in range(KD8):
                        for j in range(TT):
                            for ob in range(NB):
                                dr_mm(psums[j][ob], x8t, d, j, ob, d == 0)
                    for k in range(KT16):
                        for j in range(TT):
                            for ob in range(NB):
                                f16_mm(psums[j][ob], x16t, k, j, ob,
                                       k == KT16 - 1)
                    for j in range(TT):
                        copyback(ots[j], psums[j], t0 + j * P)
                else:
                    for j in range(TT):
                        ot = opool.tile([P, O], fp16, tag="ot",
                                        name=f"ot_{s}_{j}")
                        row = t0 + j * P
                        last = (s == NSB - 1 and j == TT - 1)
                        if last:
                            # o-block-major at half width: each block's copy
                            # + store overlaps the next block's accumulation,
                            # so only one 256-wide block's epilogue trails
                            # the PE.
                            HF = FREE // 2
                            for ob in range(2 * NB):
                                ps = psum_pool.tile(
                                    [P, HF], fp32, tag="ps",
                                    name=f"ps_{s}_{j}_{ob}")
                                for d in range(KD8):
                                    dr_mm(ps, x8t, d, j, ob, d == 0, free=HF)
                                for k in range(KT16):
                                    f16_mm(ps, x16t, k, j, ob, k == KT16 - 1,
                                           free=HF)
                                nc.scalar.mul(
                                    out=ot[:, ob * HF:(ob + 1) * HF],
                                    in_=ps,
                                    mul=gamma,
                                )
                                nc.sync.dma_start(
                                    out=out_d[row:row + P,
                                              ob * HF:(ob + 1) * HF],
                                    in_=ot[:, ob * HF:(ob + 1) * HF])
                            continue
                        psums = [psum_pool.tile([P, FREE], fp32, tag="ps",
                                                name=f"ps_{s}_{j}_{ob}")
                                 for ob in range(NB)]
                        for d in range(KD8):
                            for ob in range(NB):
                                dr_mm(psums[ob], x8t, d, j, ob, d == 0)
                        for k in range(KT16):
                            for ob in range(NB):
                                f16_mm(psums[ob], x16t, k, j, ob,
                                       k == KT16 - 1)
                        copyback(ot, psums, t0 + j * P)

    nc.compile()
    return nc


def _pack_inputs(x: np.ndarray, w: np.ndarray):
    """Host-side packing: quantize/transpose into the kernel layouts."""
    e4 = ml_dtypes.float8_e4m3fn
    NSB = N_TOKENS // SB
    # fp8 half of x: [tok, k] -> [p, s, d, i, t]
    a = x[:, :KF8].astype(e4).reshape(NSB, SB, KF8 // P, P)
    x8 = np.ascontiguousarray(a.transpose(3, 0, 2, 1)).reshape(
        P, NSB, KD8, 2, SB)
    # fp16 half of x: [tok, k] -> [p, s, k, t]
    b = x[:, KF8:].reshape(NSB, SB, KT16, P)
    x16 = np.ascontiguousarray(b.transpose(3, 0, 2, 1))
    # per-core weight shards
    w8s, w16s = [], []
    for c in range(N_CORES):
        wc = w[c * O_SHARD:(c + 1) * O_SHARD, :]  # [o, k]
        v = np.ascontiguousarray(wc[:, :KF8].T).reshape(KD8, 2, P, O_SHARD)
        w8s.append(np.ascontiguousarray(
            v.transpose(2, 0, 1, 3)).astype(e4))      # [p, d, i, o]
        v2 = np.ascontiguousarray(wc[:, KF8:].T).reshape(KT16, P, O_SHARD)
        w16s.append(np.ascontiguousarray(
            v2.transpose(1, 0, 2)).astype(e4))        # [p, k, o]
    return x8, x16, w8s, w16s


def _run(inputs, trace=False):
    import os

    from concourse.bass_utils import run_bass_kernel_spmd

    if not trace:
        # A stray BASS_TRACE would route run_bass_kernel_spmd into the NTFF
        # hook import, which this container lacks.
        os.environ["BASS_NEVER_TRACE"] = "1"
    else:
        os.environ.pop("BASS_NEVER_TRACE", None)

    x = np.asarray(inputs["x"])
    w = np.asarray(inputs["w_q"])
    gamma = float(np.asarray(inputs["gamma"]).astype(np.float32).reshape(-1)[0])

    x8, x16, w8s, w16s = _pack_inputs(x, w)
    nc = _build(gamma)
    in_maps = []
    for c in range(N_CORES):
        in_maps.append({"x8": x8, "x16": x16, "w8": w8s[c], "w16": w16s[c]})

    res = run_bass_kernel_spmd(nc, in_maps, core_ids=list(range(N_CORES)),
                               trace=trace)
    out = np.concatenate(
        [np.asarray(res.results[c]["out"]) for c in range(N_CORES)], axis=1)
    return out.astype(np.float16, copy=False), res


def kernel(**inputs) -> np.ndarray:
    out, _ = _run(inputs, trace=False)
    return out


# revision 21
# speedup vs baseline: 1.0462x; 1.0413x over previous
"""BitLinear int2 (ternary-weight) GEMM on 8 NeuronCores, fp8-hybrid.

out[8192, 16384] = (x[8192, 4096] @ w_q[16384, 4096].T) * gamma, fp16 I/O,
fp32 accumulation.

Strategy: tensor-parallel over out_features - each core gets a 2048-row
shard of w_q, x is replicated; host concatenates the 8 output shards.
The contraction is split: the first 2048 k-columns run as fp8(e4m3)
DoubleRow matmuls (2 fp8 weights per PE cell -> 256-deep contraction per
matmul, ~2x MAC rate), the last 2048 k-columns run exact fp16 matmuls.
The ternary weights are exact in e4m3; only x's fp8 half is quantized,
giving a measured 1.84e-2 relative error (gate 2e-2) at ~0.78x the
all-fp16 matmul count in PE cycles.

Both operands are host-transposed so the contraction dim lands on SBUF
partitions with plain (non-xbar) DMAs; x is host-packed per 256-token
superblock so loads are per-partition contiguous.  All weight shards
(4MB fp8 + 8MB fp16) stay resident in SBUF; x streams on the ACT HWDGE
ring while weights + outputs use the SP ring; K accumulates in PSUM
across 8 DoubleRow + 16 fp16 matmuls.  The first superblock interleaves
its two t-tiles k-outer across all 8 PSUM banks so the PE hides the
resident-weight fill; the last t-tile runs o-block-major so its copyback
trails by only one block.  gamma is baked into the PSUM->SBUF copy as an
immediate scale on the scalar engine.
"""

import sys

import ml_dtypes
import numpy as np

for _p in ("/opt/trn_rl_repo", "/root/.axon_site/_ro/trn_rl_repo"):
    if _p not in sys.path:
        sys.path.append(_p)

N_CORES = 8
N_TOKENS = 8192
IN_FEATURES = 4096
OUT_FEATURES = 16384
O_SHARD = OUT_FEATURES // N_CORES  # 2048

P = 128          # partitions / base matmul contraction tile
FREE = 512       # matmul moving free dim (one PSUM bank of fp32)
SB = 256         # tokens per x superblock (2 t-tiles)
KF8 = 2304       # leading k-columns in fp8 DoubleRow (f=9/16)
KD8 = KF8 // (2 * P)       # 9 double-slabs (256 k each)
KF16 = IN_FEATURES - KF8   # trailing k-columns in fp16
KT16 = KF16 // P           # 14 k-slabs


def _build(gamma: float, T: int = N_TOKENS, O: int = O_SHARD, sb: int = SB):
    import concourse.mybir as mybir
    from concourse import bacc
    from concourse.tile import TileContext

    fp16 = mybir.dt.float16
    fp32 = mybir.dt.float32
    fp8 = mybir.dt.float8e4
    DR = mybir.MatmulPerfMode.DoubleRow

    NB = O // FREE     # 4 o-blocks per core
    TT = sb // P       # t-tiles per superblock
    NSB = T // sb      # superblocks

    nc = bacc.Bacc("TRN2", target_bir_lowering=False, debug=False,
                   num_devices=N_CORES)
    # x fp8 half: [p, s, d, i, t] = e4m3(x[s*sb+t, (2d+i)*128+p]); per
    # partition one superblock is 4KB contiguous.
    x8_d = nc.dram_tensor("x8", (P, NSB, KD8, 2, sb), fp8,
                          kind="ExternalInput")
    # x fp16 half: [p, s, k, t] = x[s*sb+t, KF8 + k*128 + p]
    x16_d = nc.dram_tensor("x16", (P, NSB, KT16, sb), fp16,
                           kind="ExternalInput")
    # w fp8 half: [p, d, i, o] = e4m3(w[o, (2d+i)*128+p])
    w8_d2 = nc.dram_tensor("w8", (P, KD8, 2, O), fp8, kind="ExternalInput")
    # w fp16-section weights: ternary, exact in e4m3; the moving operand of
    # a normal-mode matmul may be fp8 while the stationary x stays fp16.
    # Halves the resident-weight fill bytes (8MB vs 12MB).
    # layout [p, k, o] = w[o, KF8 + k*128 + p]
    w16_d2 = nc.dram_tensor("w16", (P, KT16, O), fp8, kind="ExternalInput")
    out_d = nc.dram_tensor("out", (T, O), fp16, kind="ExternalOutput")

    with TileContext(nc) as tc:
        with tc.tile_pool(name="wpool", bufs=1) as wpool, \
             tc.tile_pool(name="x8pool", bufs=2) as x8pool, \
             tc.tile_pool(name="x16pool", bufs=2) as x16pool, \
             tc.tile_pool(name="opool", bufs=3) as opool, \
             tc.tile_pool(name="psum", bufs=8, space="PSUM") as psum_pool:

            # x loads ride the ACT HWDGE ring; weights + outputs ride the
            # SP ring so weight slab 0 is not queued behind x transfers.
            def load_x(s, eng):
                x8t = x8pool.tile([P, KD8, 2, sb], fp8, tag="x8",
                                  name=f"x8_{s}")
                if s == 0:
                    chunks = [(0, 1), (1, 2), (2, 4), (4, 6), (6, KD8)]
                else:
                    chunks = [(0, 5), (5, KD8)]
                for lo, hi in chunks:
                    eng.dma_start(out=x8t[:, lo:hi], in_=x8_d[:, s, lo:hi])
                x16t = x16pool.tile([P, KT16, sb], fp16, tag="x16",
                                    name=f"x16_{s}")
                cuts = ([0, 4, 8, 11, KT16] if s == 0 else [0, 7, KT16])
                for lo, hi in zip(cuts, cuts[1:]):
                    eng.dma_start(out=x16t[:, lo:hi],
                                  in_=x16_d[:, s, lo:hi])
                return x8t, x16t

            # PE warm-up: a few matmuls on memset scratch keep the PE busy
            # from t~0 while the first weight/x DMAs land, so the HAM clock
            # gate reaches K=8/8 by ~3.5us instead of mid-superblock-0.
            wu_l = wpool.tile([P, P], fp16, name="wu_l")
            wu_r = wpool.tile([P, FREE], fp8, name="wu_r")
            nc.gpsimd.memset(wu_l[:], 0.0)
            nc.gpsimd.memset(wu_r[:], 0.0)
            wu_ps = psum_pool.tile([P, FREE], fp32, tag="ps", name="wu_ps")
            N_WARM = 7
            for i in range(N_WARM):
                nc.tensor.matmul(wu_ps, lhsT=wu_l[:], rhs=wu_r[:],
                                 start=(i == 0), stop=(i == N_WARM - 1))

            xts = {0: load_x(0, nc.scalar)}

            # Resident weights in two mega-tiles loaded by a handful of
            # large DMAs: descriptor generation on the sync engine costs
            # ~600ns per dma_start, so many small slab loads starve the
            # early stream.  Region-granular tile deps still let the first
            # superblock's k-loop pace along the arriving chunks.  fp8
            # double-slabs first (consumed first), then fp16-section slabs.
            w8m = wpool.tile([P, KD8, 2, O], fp8, name="w8m")
            for lo, hi in [(0, 1), (1, 2), (2, 4), (4, 6), (6, KD8)]:
                nc.sync.dma_start(out=w8m[:, lo:hi], in_=w8_d2[:, lo:hi])
            w16m = wpool.tile([P, KT16, O], fp8, name="w16m")
            wcuts = [0, 4, 8, 11, KT16]
            for lo, hi in zip(wcuts, wcuts[1:]):
                nc.sync.dma_start(out=w16m[:, lo:hi], in_=w16_d2[:, lo:hi])

            # Superblock 1 queues on the SP ring *behind* the weight stream:
            # it isn't needed until ~43us and must not steal HBM bandwidth
            # from the resident-weight fill (measured: on the ACT ring it
            # runs at ~15-30us and stalls the w16 stream mid-superblock-0).
            xts[1] = load_x(1, nc.sync)

            def dr_mm(ps, x8t, d, j, ob, start, free=FREE):
                nc.tensor.matmul(
                    ps,
                    lhsT=x8t[:, d, :, j * P:(j + 1) * P],
                    rhs=w8m[:, d, :, ob * free:(ob + 1) * free],
                    start=start, stop=False, perf_mode=DR)

            def f16_mm(ps, x16t, k, j, ob, stop, free=FREE):
                nc.tensor.matmul(
                    ps,
                    lhsT=x16t[:, k, j * P:(j + 1) * P],
                    rhs=w16m[:, k, ob * free:(ob + 1) * free],
                    start=False, stop=stop)

            def copyback(ot, psums, row):
                for ob in range(NB):
                    nc.scalar.mul(
                        out=ot[:, ob * FREE:(ob + 1) * FREE],
                        in_=psums[ob],
                        mul=gamma,
                    )
                nc.sync.dma_start(out=out_d[row:row + P, :], in_=ot)

            for s in range(NSB):
                t0 = s * sb
                if s not in xts:
                    xts[s] = load_x(s, nc.scalar)
                x8t, x16t = xts[s]

                if s == 0:
                    # Interleave both t-tiles k-outer: 8 matmuls per weight
                    # slab keeps the PE behind the DMA stream during the
                    # resident-weight fill.  Uses all 8 PSUM banks.
                    ots = [opool.tile([P, O], fp16, tag="ot", name=f"ot_0_{j}")
                           for j in range(TT)]
                    psums = [[psum_pool.tile([P, FREE], fp32, tag="ps",
                                             name=f"ps_0_{j}_{ob}")
                              for ob in range(NB)] for j in range(TT)]
                    for d in range(KD8):
                        for j in range(TT):
                            for ob in range(NB):
                                dr_mm(psums[j][ob], x8t, d, j, ob, d == 0)
                    for k in range(KT16):
                        for j in range(TT):
                            for ob in range(NB):
                                f16_mm(psums[j][ob], x16t, k, j, ob,
                                       k == KT16 - 1)
                    for j in range(TT):
                        copyback(ots[j], psums[j], t0 + j * P)
                else:
                    for j in range(TT):
                        ot = opool.tile([P, O], fp16, tag="ot",
                                        name=f"ot_{s}_{j}")
                        row = t0 + j * P
                        last = (s == NSB - 1 and j == TT - 1)
                        if last:
                            # o-block-major at half width: each block's copy
                            # + store overlaps the next block's accumulation,
                            # so only one 256-wide block's epilogue trails
                            # the PE.
                            HF = FREE // 2
                            for ob in range(2 * NB):
                                ps = psum_pool.tile(
                                    [P, HF], fp32, tag="ps",
                                    name=f"ps_{s}_{j}_{ob}")
                                for d in range(KD8):
                                    dr_mm(ps, x8t, d, j, ob, d == 0, free=HF)
                                for k in range(KT16):
                                    f16_mm(ps, x16t, k, j, ob, k == KT16 - 1,
                                           free=HF)
                                nc.scalar.mul(
                                    out=ot[:, ob * HF:(ob + 1) * HF],
                                    in_=ps,
                                    mul=gamma,
                                )
                                nc.sync.dma_start(
                                    out=out_d[row:row + P,
                                              ob * HF:(ob + 1) * HF],
                                    in_=ot[:, ob * HF:(ob + 1) * HF])
                            continue
                        psums = [psum_pool.tile([P, FREE], fp32, tag="ps",
                                                name=f"ps_{s}_{j}_{ob}")
                                 for ob in range(NB)]
                        for d in range(KD8):
                            for ob in range(NB):
                                dr_mm(psums[ob], x8t, d, j, ob, d == 0)
                        for k in range(KT16):
                            for ob in range(NB):
                                f16_mm(psums[ob], x16t, k, j, ob,
                                       k == KT16 - 1)
                        copyback(ot, psums, t0 + j * P)

    nc.compile()
    return nc


def _pack_inputs(x: np.ndarray, w: np.ndarray):
    """Host-side packing: quantize/transpose into the kernel layouts."""
    e4 = ml_dtypes.float8_e4m3fn
    NSB = N_TOKENS // SB
    # fp8 half of x: [tok, k] -> [p, s, d, i, t]
    a = x[:, :KF8].astype(e4).reshape(NSB, SB, KF8 // P, P)
    x8 = np.ascontiguousarray(a.transpose(3, 0, 2, 1)).reshape(
        P, NSB, KD8, 2, SB)
    # fp16 half of x: [tok, k] -> [p, s, k, t]
    b = x[:, KF8:].reshape(NSB, SB, KT16, P)
    x16 = np.ascontiguousarray(b.transpose(3, 0, 2, 1))
    # per-core weight shards
    w8s, w16s = [], []
    for c in range(N_CORES):
        wc = w[c * O_SHARD:(c + 1) * O_SHARD, :]  # [o, k]
        v = np.ascontiguousarray(wc[:, :KF8].T).reshape(KD8, 2, P, O_SHARD)
        w8s.append(np.ascontiguousarray(
            v.transpose(2, 0, 1, 3)).astype(e4))      # [p, d, i, o]
        v2 = np.ascontiguousarray(wc[:, KF8:].T).reshape(KT16, P, O_SHARD)
        w16s.append(np.ascontiguousarray(
            v2.transpose(1, 0, 2)).astype(e4))        # [p, k, o]
    return x8, x16, w8s, w16s


def _run(inputs, trace=False):
    import os

    from concourse.bass_utils import run_bass_kernel_spmd

    if not trace:
        # A stray BASS_TRACE would route run_bass_kernel_spmd into the NTFF
        # hook import, which this container lacks.
        os.environ["BASS_NEVER_TRACE"] = "1"
    else:
        os.environ.pop("BASS_NEVER_TRACE", None)

    x = np.asarray(inputs["x"])
    w = np.asarray(inputs["w_q"])
    gamma = float(np.asarray(inputs["gamma"]).astype(np.float32).reshape(-1)[0])

    x8, x16, w8s, w16s = _pack_inputs(x, w)
    nc = _build(gamma)
    in_maps = []
    for c in range(N_CORES):
        in_maps.append({"x8": x8, "x16": x16, "w8": w8s[c], "w16": w16s[c]})

    res = run_bass_kernel_spmd(nc, in_maps, core_ids=list(range(N_CORES)),
                               trace=trace)
    out = np.concatenate(
        [np.asarray(res.results[c]["out"]) for c in range(N_CORES)], axis=1)
    return out.astype(np.float16, copy=False), res


def kernel(**inputs) -> np.ndarray:
    out, _ = _run(inputs, trace=False)
    return out
